# revision 23
# baseline (speedup 1.0000x reference)
"""Trainium2 Bass kernel for dual cross-attention + mean-fuse MLP (CAFM).

Problem: B=16, C=256, H*W=N=2048, DIM=256.
  out_1 = cross_attn(stft_seq, cqt_seq, wq1, wq2, wq3)   # [B, N, C]
  out_2 = cross_attn(cqt_seq, stft_seq, wq4, wq5, wq6)
  fused = concat([mean_n(out_1), mean_n(out_2)])         # [B, 512]
  out   = relu(fused @ W1 + b1) @ W2 + b2                # [B, 256]

Key algebra (exact):
  * softmax is invariant to per-row constants, so
      S = (X Wq + bq)(Y Wk + bk)^T * s  ~  (X A + 1 w^T) Y^T
    with A = s Wq Wk^T, w = s Wk bq — the K projection disappears.
  * only mean_n(softmax(S) V) is needed, so instead of full attn @ V:
      y = p^T V + bv,  p[m] = (1/N) sum_n exp(S[n,m]) / rowsum[n]
    via rinv-weighted reduction matmuls per 128-row block.

Engine split (the exp over N^2 elements is the wall — Act alone is
~153G elem/s, so exp is split across Act and DVE):
  * scores in fp8 DoubleRow (0.5 cyc/row) into a 6-bank PSUM ring.
  * 10 of 16 row-blocks: Act native Exp -> fp8e4 e (+fused rowsum).
  * 6 of 16 row-blocks (pairs {1,3,5}): DVE custom op EXPBITS16 —
    Schraudolph bits trick: round(x*log2e*128 + (127+a)*128) via
    magic-constant add, written through f32->int16 conversion; the
    int16 IS the bf16 bit pattern of exp(x). Rowsum via DVE
    tensor_reduce (bf16, 2x mode).
  * rinv = 1-Newton bit-seed reciprocal (custom DVE op, x256) -> fp8/bf16.
  * column sums p: PE matmuls — fp8 DR block-pairs for Act blocks,
    bf16 for DVE blocks — into one PSUM bank (4 chunk accumulators at
    partitions 0/32/64/96 via matmul tile_position).
  * T~ evac: Act Identity+bias (fp8 out). V evac: gpsimd. p evac: DMA.

Sharding: data-parallel over batch, 2 batch elements per core, both
attention directions per core. No collectives.
"""

import numpy as np

import concourse.bass as bass
import concourse.mybir as mybir
import concourse.tile as tile
from concourse.bass_utils import run_bass_kernel_spmd

F32 = mybir.dt.float32
F32R = mybir.dt.float32r
FP8 = mybir.dt.float8e4
BF16 = mybir.dt.bfloat16
I16 = mybir.dt.int16
DR = mybir.MatmulPerfMode.DoubleRow
AF = mybir.ActivationFunctionType

N = 2048          # sequence length (H*W)
C = 256           # channels
BLOCKS = N // 128  # 16 row blocks
D_COLS = {(0, 1), (1, 1), (2, 1), (3, 1), (5, 1), (7, 1)}  # (pair, pc) on DVE

# Schraudolph constants: scores arrive pre-scaled by 128*log2e (folded
# into the host-side A/w and k8 fp8 scalings), so the DVE exp is ONE
# native tensor_scalar_add with int16 output (the f32->int16 write
# conversion rounds; the int16 IS the bf16 bit pattern of exp(x)).
EXP_SCALE = float(128.0 * np.log2(np.e))
EXP_BIAS = float((127.0 + 0.057) * 128.0)
TT_HOST = 160.0                   # host factor on A, w (fp8 |tt| <~ 20)
K_CAST = float(EXP_SCALE / TT_HOST)  # fp8 cast scalar for the kv side
RCP_SCALE = 256.0

# ---------------------------------------------------------------------------
# Custom DVE ops (registered into concourse.dve_ops at import time).
# ---------------------------------------------------------------------------


def _register_custom_ops():
    from concourse import dve_ops
    from concourse.dve_spec import (
        Spec, Src0, Src1, C0, C1, C2, Bin, lower, _has_src1)
    from concourse.dve_uop import AluOp
    from concourse.dve_table_gen import DveOpSpec

    if "EXPBITS16_ANT" in dve_ops._SUB_OPCODE_FOR_NAME:
        return (dve_ops._BY_NAME_ANT["EXPBITS16_ANT"],
                dve_ops._BY_NAME_ANT["RECIPSUM_ANT"])

    def _ref_expbits16(in0, in1, c0, c1, c2):
        t = in0.astype(np.float32) * np.float32(c0)
        m = (t + np.float32(c1)).astype(np.float32)
        return (m - np.float32(c2)).astype(np.float32)

    exp_spec = Spec(body=(Src0 * C0 + C1) - C2, reference=_ref_expbits16)

    def _ref_recipsum(in0, in1, c0, c1, c2):
        x = (in0.astype(np.float32) + in1.astype(np.float32))
        nx = (~x.view(np.int32)).view(np.float32)
        y0 = nx * np.float32(c0)
        y1 = y0 * (np.float32(c1) - x * y0)
        return y1 * np.float32(c2)

    _x = Src0 + Src1
    _nx = Bin(AluOp.BITWISE_NOT, _x, _x)
    _y0 = _nx * C0
    _y1 = _y0 * (C1 - _x * _y0)
    rcp_spec = Spec(body=_y1 * C2, reference=_ref_recipsum)

    ops = []
    for name, spec in (("EXPBITS16_ANT", exp_spec), ("RECIPSUM_ANT", rcp_spec)):
        row = dve_ops._CUSTOM_DVE_ROW_BASE + len(dve_ops.OPS)
        dve_ops._SUB_OPCODE_FOR_NAME[name] = row
        sha = {}
        for ver in ("v3", "v4"):
            uops = lower(spec, ver=ver)
            sha[ver] = DveOpSpec(
                name=name, opcode=row, uops=uops,
                rd1_en=_has_src1(spec)).sha(ver)
        op = dve_ops.DveOp(name, spec, subdim=False, uops_sha=sha)
        dve_ops.OPS.append(op)
        dve_ops.CUSTOM_DVE_SPECS[name] = spec
        ops.append(op)
    dve_ops._BY_NAME_ANT = {o.name: o for o in ops}
    return tuple(ops)


EXP_OP, RCP_OP = None, None


def _ops():
    global EXP_OP, RCP_OP
    if EXP_OP is None:
        EXP_OP, RCP_OP = _register_custom_ops()
    return EXP_OP, RCP_OP


def split_multi_waits(nc):
    """This container's walrus accepts at most 1 sync-wait per instruction
    (2 for EventSemaphore). Tile's tail drain can carry more; move the
    excess onto preceding wait-only NoOps on the same engine."""
    f = nc.m.functions[0]
    n_new = 0
    for bb in f.blocks:
        insts = bb.instructions
        new_list = []
        changed = False
        for inst in insts:
            si = inst.sync_info
            waits = list(si.on_wait) if si and si.on_wait else []
            cap = 2 if isinstance(inst, mybir.InstEventSemaphore) else 1
            if len(waits) > cap:
                for w in waits[:-cap]:
                    nop = mybir.InstNoOp(
                        name=f"I-sw{n_new}-{inst.name}", ins=[], outs=[])
                    n_new += 1
                    nop.engine = inst.engine
                    nop.sync_info = mybir.SyncInfo(on_wait=[w], on_update=[])
                    new_list.append(nop)
                si.on_wait = waits[-cap:]
                inst.sync_info = si
                changed = True
            new_list.append(inst)
        if changed:
            bb.instructions = new_list
    return n_new


def build_nc(reps=1):
    nc = bass.Bass("TRN2", target_bir_lowering=False, debug=False)

    # --- DRAM I/O (per core) ---
    xq_d = nc.dram_tensor("xq", [2, C, N], F32, kind="ExternalInput")  # stft
    xk_d = nc.dram_tensor("xk", [2, C, N], F32, kind="ExternalInput")  # cqt
    a_d = [nc.dram_tensor(f"a{d}", [C, C], F32, kind="ExternalInput")
           for d in range(2)]
    wt_d = [nc.dram_tensor(f"wt{d}", [C], F32, kind="ExternalInput")
            for d in range(2)]
    wv_d = [nc.dram_tensor(f"wv{d}", [C, C], F32, kind="ExternalInput")
            for d in range(2)]
    bv_d = [nc.dram_tensor(f"bv{d}", [C], F32, kind="ExternalInput")
            for d in range(2)]
    w1_d = nc.dram_tensor("w1", [2 * C, C], F32, kind="ExternalInput")
    b1_d = nc.dram_tensor("b1", [C], F32, kind="ExternalInput")
    w2_d = nc.dram_tensor("w2", [C, C], F32, kind="ExternalInput")
    b2_d = nc.dram_tensor("b2", [C], F32, kind="ExternalInput")
    out_d = nc.dram_tensor("out", [C, 2], F32, kind="ExternalOutput")

    with tile.TileContext(nc) as tc, nc.allow_low_precision(reason="f32r/fp8"):
        with (
            tc.tile_pool(name="const", bufs=1) as const,
            tc.tile_pool(name="seq", bufs=1) as seqp,
            tc.tile_pool(name="tt", bufs=1) as ttp,
            tc.tile_pool(name="vv", bufs=1) as vvp,
            tc.tile_pool(name="ee", bufs=2) as eep,
            tc.tile_pool(name="small", bufs=1) as smallp,
            tc.tile_pool(name="s6", bufs=1, space="PSUM") as s6p,
            tc.tile_pool(name="pk", bufs=1, space="PSUM") as pkp,
            tc.tile_pool(name="scr", bufs=1, space="PSUM") as scrp,
        ):
            # --- sequence + weight loads (d=0 weights first: needed early) ---
            xq_sbs = [seqp.tile([128, 2, N], F32R, tag=f"xq{b}",
                                name=f"xq_sb{b}") for b in range(2)]
            xk_sbs = [seqp.tile([128, 2, N], F32R, tag=f"xk{b}",
                                name=f"xk_sb{b}") for b in range(2)]
            one_sb = const.tile([128, 1], F32)
            nc.vector.memset(one_sb, 1.0)
            zero1 = const.tile([128, 1], F32)
            nc.vector.memset(zero1, 0.0)

            a_sb, wt_sb, wv_sb, bv_sb = [], [], [], []
            for d in range(2):
                a = const.tile([128, 2, C], F32R, tag=f"a{d}")
                nc.sync.dma_start(
                    out=a,
                    in_=a_d[d].ap().rearrange("(k p) c -> p k c", p=128).bitcast(F32R))
                a_sb.append(a)
                wt = const.tile([128, 2], F32, tag=f"wt{d}")
                nc.sync.dma_start(
                    out=wt, in_=wt_d[d].ap().rearrange("(t p) -> p t", p=128))
                wt_sb.append(wt)
                wv = const.tile([128, 2, C], F32R, tag=f"wv{d}")
                nc.scalar.dma_start(
                    out=wv,
                    in_=wv_d[d].ap().rearrange("(k p) c -> p k c", p=128).bitcast(F32R))
                wv_sb.append(wv)
                bv = const.tile([1, C], F32, tag=f"bv{d}")
                nc.scalar.dma_start(
                    out=bv, in_=bv_d[d].ap().rearrange("(o c) -> o c", o=1))
                bv_sb.append(bv)
                if d == 0:
                    nc.sync.dma_start(
                        out=xq_sbs[0],
                        in_=xq_d.ap()[0].rearrange(
                            "(k p) n -> p k n", p=128).bitcast(F32R))
                    nc.scalar.dma_start(
                        out=xk_sbs[0],
                        in_=xk_d.ap()[0].rearrange(
                            "(k p) n -> p k n", p=128).bitcast(F32R))
                    nc.sync.dma_start(
                        out=xq_sbs[1],
                        in_=xq_d.ap()[1].rearrange(
                            "(k p) n -> p k n", p=128).bitcast(F32R))
                    nc.scalar.dma_start(
                        out=xk_sbs[1],
                        in_=xk_d.ap()[1].rearrange(
                            "(k p) n -> p k n", p=128).bitcast(F32R))

            w1_sb = const.tile([128, 4, C], F32)
            nc.sync.dma_start(
                out=w1_sb, in_=w1_d.ap().rearrange("(k p) c -> p k c", p=128))
            b1_sb = const.tile([128, 2], F32)
            nc.sync.dma_start(
                out=b1_sb, in_=b1_d.ap().rearrange("(t p) -> p t", p=128))
            w2_sb = const.tile([128, 2, C], F32)
            nc.scalar.dma_start(
                out=w2_sb, in_=w2_d.ap().rearrange("(k p) c -> p k c", p=128))
            b2_sb = const.tile([128, 2], F32)
            nc.scalar.dma_start(
                out=b2_sb, in_=b2_d.ap().rearrange("(t p) -> p t", p=128))

            # fp8 copies of the sequences (kv-side scores operand), x1/16.
            xq8s = [seqp.tile([128, 2, N], FP8, tag=f"xq8{b}",
                              name=f"xq8_{b}") for b in range(2)]
            xk8s = [seqp.tile([128, 2, N], FP8, tag=f"xk8{b}",
                              name=f"xk8_{b}") for b in range(2)]
            nc.vector.tensor_scalar_mul(
                xk8s[0], xk_sbs[0].bitcast(F32), K_CAST)
            nc.gpsimd.tensor_scalar_mul(
                xq8s[0], xq_sbs[0].bitcast(F32), K_CAST)
            nc.gpsimd.tensor_scalar_mul(
                xk8s[1], xk_sbs[1].bitcast(F32), K_CAST)
            nc.gpsimd.tensor_scalar_mul(
                xq8s[1], xq_sbs[1].bitcast(F32), K_CAST)

            ft_sb = const.tile([128, 8], F32)  # fused^T columns (k-chunk, b)

            # scores ring: three 2-bank tiles; pieces cycle tiles by
            # global half-block index so PE writes ~1.5 blocks ahead of exp
            s3 = [s6p.tile([128, 1024], F32, tag=f"s{k}", name=f"s3_{k}")
                  for k in range(3)]

            seq_idx = [(r, b, d) for r in range(reps)
                       for b in range(2) for d in range(2)]
            n_idx = len(seq_idx)

            # ---- startup: T~ and V for the four (b,d) combos (both are
            # rep-invariant, so no per-rep PSUM traffic or evacuation) ----
            tt_tiles = {}
            v_tiles = {}
            for b in range(2):
                for d in range(2):
                    q = xq_sbs[b] if d == 0 else xk_sbs[b]
                    kv = xk_sbs[b] if d == 0 else xq_sbs[b]
                    aa, ww, wvv = a_sb[d], wt_sb[d], wv_sb[d]
                    t = ttp.tile([128, 2, N], FP8, tag=f"tt{b}{d}",
                                 name=f"tt{b}{d}")
                    tt_tiles[(b, d)] = t
                    v = vvp.tile([128, BLOCKS, C], BF16, tag=f"v{b}{d}",
                                 name=f"v{b}{d}")
                    v_tiles[(b, d)] = v
                    for ct in range(2):
                        for j4 in range(4):
                            lo = 512 * j4
                            ps = (pkp if j4 % 2 == 0 else scrp).tile(
                                [128, 512], F32,
                                tag="pacc" if j4 % 2 == 0 else "scr0",
                                name=f"ttps{b}{d}_{ct}{j4}")
                            nc.tensor.matmul(
                                ps, aa[:, 0, ct * 128:(ct + 1) * 128],
                                q[:, 0, lo:lo + 512], start=True, stop=False)
                            nc.tensor.matmul(
                                ps, aa[:, 1, ct * 128:(ct + 1) * 128],
                                q[:, 1, lo:lo + 512], start=False, stop=True)
                            if j4 % 2 == 0:
                                nc.scalar.activation(
                                    t[:, ct, lo:lo + 512], ps, AF.Identity,
                                    bias=ww[:, ct:ct + 1], scale=1.0)
                            else:
                                nc.vector.tensor_scalar_add(
                                    t[:, ct, lo:lo + 512], ps,
                                    ww[:, ct:ct + 1])
                    for mb in range(BLOCKS):
                        ps = (pkp if mb % 2 == 0 else scrp).tile(
                            [128, 512], F32,
                            tag="pacc" if mb % 2 == 0 else "scr0",
                            name=f"vps{b}{d}_{mb}")
                        nc.tensor.matmul(
                            ps[:, :C], kv[:, 0, mb * 128:(mb + 1) * 128],
                            wvv[:, 0, :], start=True, stop=False)
                        nc.tensor.matmul(
                            ps[:, :C], kv[:, 1, mb * 128:(mb + 1) * 128],
                            wvv[:, 1, :], start=False, stop=True)
                        if mb % 2 == 0:
                            nc.scalar.activation(
                                v[:, mb, :], ps[:, :C], AF.Identity)
                        else:
                            nc.vector.tensor_copy(v[:, mb, :], ps[:, :C])

            pair_tiles = {}   # (gi, pair) -> (e0, e1, rv)
            block_data = {}   # g -> [partial, partial]
            paccs = {}        # gi -> (chunk tiles, started[4])

            def ctx(gi):
                _, b, d = seq_idx[gi]
                k8 = xk8s[b] if d == 0 else xq8s[b]
                kv = xk_sbs[b] if d == 0 else xq_sbs[b]
                return b, d, k8, kv

            def scores_half(g, half):
                gi, nb = divmod(g, BLOCKS)
                _, b, d = seq_idx[gi]
                _, _, k8, _ = ctx(gi)
                tt = tt_tiles[(b, d)]
                for ci in ((0, 1) if half == 0 else (2, 3)):
                    t3 = s3[(2 * g + ci // 2) % 3]
                    nc.tensor.matmul(
                        t3[:, (ci % 2) * 512:(ci % 2 + 1) * 512],
                        tt[:, :, nb * 128:(nb + 1) * 128],
                        k8[:, :, ci * 512:(ci + 1) * 512],
                        start=True, stop=True, perf_mode=DR)

            def emit_exp(g):
                gi, nb = divmod(g, BLOCKS)
                pair, par = nb >> 1, nb & 1
                key = (gi, pair)
                if par == 0:
                    e0 = eep.tile([128, 2, 1024], BF16, tag="e16a",
                                  name=f"e16a_{gi}_{pair}", bufs=3)
                    if (pair, 1) in D_COLS:
                        e1 = eep.tile([128, 2, 1024], I16, tag="e16d",
                                      name=f"e16d_{gi}_{pair}", bufs=3)
                    else:
                        e1 = eep.tile([128, 2, 1024], BF16, tag="e16b",
                                      name=f"e16b_{gi}_{pair}", bufs=3)
                    rv_t = smallp.tile([128, 2, 1], BF16, tag="rv16", bufs=4)
                    pair_tiles[key] = (e0, e1, rv_t)
                e0, e1, rv_t = pair_tiles[key]

                parts = []
                for pc, e_t in ((0, e0), (1, e1)):
                    sc = s3[(2 * g + pc) % 3]
                    if (pair, pc) not in D_COLS:
                        ra = smallp.tile([128, 1], F32, tag=f"ra{pc}", bufs=6)
                        nc.scalar.activation(
                            e_t[:, par, :], sc, AF.Exp,
                            scale=1.0 / EXP_SCALE, accum_out=ra)
                    else:
                        nc.vector.tensor_scalar_add(
                            e_t[:, par, :], sc, EXP_BIAS)
                        t1 = smallp.tile([128, 512], BF16, tag="dr1", bufs=3)
                        nc.gpsimd.tensor_add(
                            t1, e_t[:, par, 0:512].bitcast(BF16),
                            e_t[:, par, 512:1024].bitcast(BF16))
                        t2 = smallp.tile([128, 256], BF16, tag="dr2", bufs=3)
                        nc.gpsimd.tensor_add(
                            t2, t1[:, 0:256], t1[:, 256:512])
                        ra = smallp.tile([128, 1], F32, tag=f"ra{pc}", bufs=6)
                        nc.vector.tensor_reduce(
                            ra, t2, axis=mybir.AxisListType.X,
                            op=mybir.AluOpType.add)
                    parts.append(ra)
                block_data[g] = parts

            def emit_colsum(gi, pair):
                e0, e1, rv_t = pair_tiles.pop((gi, pair))
                if gi not in paccs:
                    paccs[gi] = (pkp.tile([128, 512], F32, tag="pacc",
                                          name=f"pacc{gi}"), [False] * 4)
                pacc, started = paccs[gi]
                for par in range(2):
                    pa, pb = block_data.pop(BLOCKS * gi + 2 * pair + par)
                    rs = smallp.tile([128, 1], F32, tag="rs", bufs=4)
                    nc.gpsimd.tensor_add(rs, pa, pb)
                    nc.vector.reciprocal(rv_t[:, par, :], rs)
                last = pair == BLOCKS // 2 - 1
                for ci in range(4):
                    st = not started[ci]
                    started[ci] = True
                    pc, off = ci // 2, (ci % 2) * 512
                    e_t = (e0, e1)[pc]
                    dd = (pair, pc) in D_COLS
                    for pp in range(2):
                        nc.tensor.matmul(
                            pacc[32 * ci:32 * ci + 1, :],
                            rv_t[:, pp, :],
                            e_t[:, pp, off:off + 512].bitcast(BF16)
                            if dd else e_t[:, pp, off:off + 512],
                            start=st and pp == 0,
                            stop=last and pp == 1,
                            tile_position=(0, 32 * ci),
                            skip_group_check=True)

            tail_state = {}

            def tail_p(gi):
                pacc, _ = paccs.pop(gi)
                p_sb = smallp.tile([1, 4, 512], F32, tag="p",
                                   name=f"p{gi}", bufs=2)
                for ci in range(4):
                    if ci % 2 == 0:
                        nc.scalar.activation(
                            p_sb[:, ci, :], pacc[32 * ci:32 * ci + 1, :],
                            AF.Identity)
                    else:
                        nc.vector.tensor_copy(
                            p_sb[:, ci, :], pacc[32 * ci:32 * ci + 1, :])
                tail_state[gi] = p_sb

            def tail_ptp(gi):
                p_sb = tail_state[gi]
                ptp = scrp.tile([128, 512], F32, tag="scr0", name=f"ptp{gi}")
                for j in range(BLOCKS):
                    nc.tensor.matmul(
                        ptp[:, j:j + 1],
                        p_sb[0:1, j // 4, (j % 4) * 128:(j % 4 + 1) * 128],
                        one_sb[0:1, :], start=(j == 0),
                        stop=(j == BLOCKS - 1), skip_group_check=True)
                pt_sb = smallp.tile([128, 16], BF16, tag="pt", bufs=2)
                nc.vector.tensor_copy(pt_sb, ptp[:, :16])
                tail_state[gi] = pt_sb

            def tail_yps(gi):
                pt_sb = tail_state[gi]
                b, d, _, _ = ctx(gi)
                v = v_tiles[(b, d)]
                yps = scrp.tile([128, 512], F32, tag="scr0", name=f"yps{gi}")
                for j in range(BLOCKS):
                    nc.tensor.matmul(
                        yps[0:1, :C], pt_sb[:, j:j + 1], v[:, j, :],
                        start=(j == 0), stop=(j == BLOCKS - 1),
                        skip_group_check=True)
                y_sb = smallp.tile([1, C], F32, tag="y", bufs=2)
                nc.vector.tensor_add(y_sb, yps[0:1, :C], bv_sb[d])
                tail_state[gi] = y_sb

            def tail_fin(gi):
                y_sb = tail_state.pop(gi)
                _, b, d = seq_idx[gi]
                fcol = scrp.tile([128, 512], F32, tag="scr0", name=f"fcol{gi}")
                for h in range(2):
                    nc.tensor.matmul(
                        fcol[:, h:h + 1], y_sb[0:1, h * 128:(h + 1) * 128],
                        one_sb[0:1, :], start=(h == 0), stop=(h == 1),
                        skip_group_check=True)
                for h in range(2):
                    k = 2 * d + h
                    nc.vector.tensor_copy(
                        ft_sb[:, 2 * k + b:2 * k + b + 1], fcol[:, h:h + 1])

            schedule = {}

            def at(git, fn):
                schedule.setdefault(git, []).append(fn)

            for gi in range(n_idx):
                for p in range(8):
                    at(BLOCKS * gi + 2 * p + 5,
                       lambda gi=gi, p=p: emit_colsum(gi, p))
                base = BLOCKS * gi + BLOCKS + 3
                at(base, lambda gi=gi: tail_p(gi))
                at(base + 1, lambda gi=gi: tail_ptp(gi))
                at(base + 2, lambda gi=gi: tail_yps(gi))
                at(base + 3, lambda gi=gi: tail_fin(gi))

            total = BLOCKS * n_idx
            for git in range(total + 7):
                if git < total:
                    scores_half(git, 0)
                for fn in schedule.pop(git, []):
                    fn()
                if git < total:
                    scores_half(git, 1)
                if 1 <= git <= total:
                    emit_exp(git - 1)

            # --- final MLP on the two local batch rows ---
            h_sb = smallp.tile([128, 2, 2], F32, tag="h")
            for t in range(2):
                hps = scrp.tile([128, 512], F32, tag="scr0", name=f"hps{t}")
                for k in range(4):
                    nc.tensor.matmul(
                        hps[:, 0:2], w1_sb[:, k, t * 128:(t + 1) * 128],
                        ft_sb[:, 2 * k:2 * k + 2],
                        start=(k == 0), stop=(k == 3), skip_group_check=True)
                nc.scalar.activation(
                    h_sb[:, t, :], hps[:, 0:2], AF.Relu,
                    bias=b1_sb[:, t:t + 1], scale=1.0)
            o_sb = smallp.tile([128, 2, 2], F32, tag="o")
            for t in range(2):
                ops = pkp.tile([128, 512], F32, tag="pacc", name=f"ops{t}")
                for k in range(2):
                    nc.tensor.matmul(
                        ops[:, 0:2], w2_sb[:, k, t * 128:(t + 1) * 128],
                        h_sb[:, k, :],
                        start=(k == 0), stop=(k == 1), skip_group_check=True)
                nc.scalar.activation(
                    o_sb[:, t, :], ops[:, 0:2], AF.Identity,
                    bias=b2_sb[:, t:t + 1], scale=1.0)
            nc.sync.dma_start(
                out=out_d.ap().rearrange("(t p) b -> p t b", p=128), in_=o_sb)

    split_multi_waits(nc)
    return nc


_NC = None


def _get_nc():
    global _NC
    if _NC is None:
        _NC = build_nc()
    return _NC


def prep_inputs(stft_feat, cqt_feat, wq1_w, wq1_b, wq2_w, wq2_b, wq3_w, wq3_b,
                wq4_w, wq4_b, wq5_w, wq5_b, wq6_w, wq6_b,
                out1_w, out1_b, out2_w, out2_b):
    B = stft_feat.shape[0]
    s = 1.0 / np.sqrt(np.float32(C))
    f32 = np.float32
    sigma = np.float32(TT_HOST)  # fp8 range balancing; scores come out
    # scaled by EXP_SCALE = TT_HOST * K_CAST (Schraudolph pre-scale)
    A1 = (wq1_w @ wq2_w.T * s * sigma).astype(f32)
    wt1 = (wq2_w @ wq1_b * s * sigma).astype(f32)
    A2 = (wq4_w @ wq5_w.T * s * sigma).astype(f32)
    wt2 = (wq5_w @ wq4_b * s * sigma).astype(f32)
    WV1 = (wq3_w / f32(N)).astype(f32)
    WV2 = (wq6_w / f32(N)).astype(f32)
    common = dict(
        a0=np.ascontiguousarray(A1), a1=np.ascontiguousarray(A2),
        wt0=np.ascontiguousarray(wt1), wt1=np.ascontiguousarray(wt2),
        wv0=np.ascontiguousarray(WV1), wv1=np.ascontiguousarray(WV2),
        bv0=np.ascontiguousarray(wq3_b.astype(f32)),
        bv1=np.ascontiguousarray(wq6_b.astype(f32)),
        w1=np.ascontiguousarray(out1_w.astype(f32)),
        b1=np.ascontiguousarray(out1_b.astype(f32)),
        w2=np.ascontiguousarray(out2_w.astype(f32)),
        b2=np.ascontiguousarray(out2_b.astype(f32)),
    )
    stft = np.ascontiguousarray(stft_feat.reshape(B, C, N).astype(f32))
    cqt = np.ascontiguousarray(cqt_feat.reshape(B, C, N).astype(f32))
    in_maps = []
    for i in range(8):
        m = dict(common)
        m["xq"] = np.ascontiguousarray(stft[2 * i:2 * i + 2])
        m["xk"] = np.ascontiguousarray(cqt[2 * i:2 * i + 2])
        in_maps.append(m)
    return in_maps


def kernel(**inputs):
    inputs = {k: np.asarray(v) for k, v in inputs.items()}
    B = inputs["stft_feat"].shape[0]
    nc = _get_nc()
    in_maps = prep_inputs(**inputs)
    res = run_bass_kernel_spmd(nc, in_maps, list(range(8)))
    out = np.empty((B, C), np.float32)
    for i in range(8):
        o = res.results[i]["out"]  # [C, 2]
        out[2 * i] = o[:, 0]
        out[2 * i + 1] = o[:, 1]
    return out


# revision 30
# speedup vs baseline: 1.2721x; 1.2721x over previous
"""Trainium2 Bass kernel for dual cross-attention + mean-fuse MLP (CAFM).

Problem: B=16, C=256, H*W=N=2048, DIM=256.
  out_1 = cross_attn(stft_seq, cqt_seq, wq1, wq2, wq3)   # [B, N, C]
  out_2 = cross_attn(cqt_seq, stft_seq, wq4, wq5, wq6)
  fused = concat([mean_n(out_1), mean_n(out_2)])         # [B, 512]
  out   = relu(fused @ W1 + b1) @ W2 + b2                # [B, 256]

Key algebra (exact):
  * softmax is invariant to per-row constants, so
      S = (X Wq + bq)(Y Wk + bk)^T * s  ~  (X A + 1 w^T) Y^T
    with A = s Wq Wk^T, w = s Wk bq — the K projection disappears.
  * only mean_n(softmax(S) V) is needed, so instead of full attn @ V:
      y = p^T V + bv,  p[m] = (1/N) sum_n exp(S[n,m]) / rowsum[n]
    via rinv-weighted reduction matmuls per 128-row block.

Engine split (the exp over N^2 elements is the wall — Act alone is
~153G elem/s, so exp is split across Act and DVE):
  * scores in fp8 DoubleRow (0.5 cyc/row) into a 6-bank PSUM ring.
  * 10 of 16 row-blocks: Act native Exp -> fp8e4 e (+fused rowsum).
  * 6 of 16 row-blocks (pairs {1,3,5}): DVE custom op EXPBITS16 —
    Schraudolph bits trick: round(x*log2e*128 + (127+a)*128) via
    magic-constant add, written through f32->int16 conversion; the
    int16 IS the bf16 bit pattern of exp(x). Rowsum via DVE
    tensor_reduce (bf16, 2x mode).
  * rinv = 1-Newton bit-seed reciprocal (custom DVE op, x256) -> fp8/bf16.
  * column sums p: PE matmuls — fp8 DR block-pairs for Act blocks,
    bf16 for DVE blocks — into one PSUM bank (4 chunk accumulators at
    partitions 0/32/64/96 via matmul tile_position).
  * T~ evac: Act Identity+bias (fp8 out). V evac: gpsimd. p evac: DMA.

Sharding: data-parallel over batch, 2 batch elements per core, both
attention directions per core. No collectives.
"""

import numpy as np

import concourse.bass as bass
import concourse.mybir as mybir
import concourse.tile as tile
from concourse.bass_utils import run_bass_kernel_spmd

F32 = mybir.dt.float32
F32R = mybir.dt.float32r
FP8 = mybir.dt.float8e4
BF16 = mybir.dt.bfloat16
I16 = mybir.dt.int16
DR = mybir.MatmulPerfMode.DoubleRow
AF = mybir.ActivationFunctionType

N = 2048          # sequence length (H*W)
C = 256           # channels
BLOCKS = N // 128  # 16 row blocks
D_COLS = {(0, 1), (1, 1), (2, 1), (3, 1), (5, 1), (7, 1)}  # (pair, pc) on DVE

# Schraudolph constants: scores arrive pre-scaled by 128*log2e (folded
# into the host-side A/w and k8 fp8 scalings), so the DVE exp is ONE
# native tensor_scalar_add with int16 output (the f32->int16 write
# conversion rounds; the int16 IS the bf16 bit pattern of exp(x)).
EXP_SCALE = float(128.0 * np.log2(np.e))
EXP_BIAS = float((127.0 + 0.057) * 128.0)
TT_HOST = 160.0                   # host factor on A, w (fp8 |tt| <~ 20)
K_CAST = float(EXP_SCALE / TT_HOST)  # fp8 cast scalar for the kv side
RCP_SCALE = 256.0

# ---------------------------------------------------------------------------
# Custom DVE ops (registered into concourse.dve_ops at import time).
# ---------------------------------------------------------------------------


def _register_custom_ops():
    from concourse import dve_ops
    from concourse.dve_spec import (
        Spec, Src0, Src1, C0, C1, C2, Bin, lower, _has_src1)
    from concourse.dve_uop import AluOp
    from concourse.dve_table_gen import DveOpSpec

    if "EXPBITS16_ANT" in dve_ops._SUB_OPCODE_FOR_NAME:
        return (dve_ops._BY_NAME_ANT["EXPBITS16_ANT"],
                dve_ops._BY_NAME_ANT["RECIPSUM_ANT"])

    def _ref_expbits16(in0, in1, c0, c1, c2):
        t = in0.astype(np.float32) * np.float32(c0)
        m = (t + np.float32(c1)).astype(np.float32)
        return (m - np.float32(c2)).astype(np.float32)

    exp_spec = Spec(body=(Src0 * C0 + C1) - C2, reference=_ref_expbits16)

    def _ref_recipsum(in0, in1, c0, c1, c2):
        x = (in0.astype(np.float32) + in1.astype(np.float32))
        nx = (~x.view(np.int32)).view(np.float32)
        y0 = nx * np.float32(c0)
        y1 = y0 * (np.float32(c1) - x * y0)
        return y1 * np.float32(c2)

    _x = Src0 + Src1
    _nx = Bin(AluOp.BITWISE_NOT, _x, _x)
    _y0 = _nx * C0
    _y1 = _y0 * (C1 - _x * _y0)
    rcp_spec = Spec(body=_y1 * C2, reference=_ref_recipsum)

    ops = []
    for name, spec in (("EXPBITS16_ANT", exp_spec), ("RECIPSUM_ANT", rcp_spec)):
        row = dve_ops._CUSTOM_DVE_ROW_BASE + len(dve_ops.OPS)
        dve_ops._SUB_OPCODE_FOR_NAME[name] = row
        sha = {}
        for ver in ("v3", "v4"):
            uops = lower(spec, ver=ver)
            sha[ver] = DveOpSpec(
                name=name, opcode=row, uops=uops,
                rd1_en=_has_src1(spec)).sha(ver)
        op = dve_ops.DveOp(name, spec, subdim=False, uops_sha=sha)
        dve_ops.OPS.append(op)
        dve_ops.CUSTOM_DVE_SPECS[name] = spec
        ops.append(op)
    dve_ops._BY_NAME_ANT = {o.name: o for o in ops}
    return tuple(ops)


EXP_OP, RCP_OP = None, None


def _ops():
    global EXP_OP, RCP_OP
    if EXP_OP is None:
        EXP_OP, RCP_OP = _register_custom_ops()
    return EXP_OP, RCP_OP


def split_multi_waits(nc):
    """This container's walrus accepts at most 1 sync-wait per instruction
    (2 for EventSemaphore). Tile's tail drain can carry more; move the
    excess onto preceding wait-only NoOps on the same engine."""
    f = nc.m.functions[0]
    n_new = 0
    for bb in f.blocks:
        insts = bb.instructions
        new_list = []
        changed = False
        for inst in insts:
            si = inst.sync_info
            waits = list(si.on_wait) if si and si.on_wait else []
            cap = 2 if isinstance(inst, mybir.InstEventSemaphore) else 1
            if len(waits) > cap:
                for w in waits[:-cap]:
                    nop = mybir.InstNoOp(
                        name=f"I-sw{n_new}-{inst.name}", ins=[], outs=[])
                    n_new += 1
                    nop.engine = inst.engine
                    nop.sync_info = mybir.SyncInfo(on_wait=[w], on_update=[])
                    new_list.append(nop)
                si.on_wait = waits[-cap:]
                inst.sync_info = si
                changed = True
            new_list.append(inst)
        if changed:
            bb.instructions = new_list
    return n_new


def build_nc(reps=1):
    nc = bass.Bass("TRN2", target_bir_lowering=False, debug=False)

    # --- DRAM I/O (per core) ---
    xq_d = nc.dram_tensor("xq", [2, C, N], F32, kind="ExternalInput")  # stft
    xk_d = nc.dram_tensor("xk", [2, C, N], F32, kind="ExternalInput")  # cqt
    a_d = [nc.dram_tensor(f"a{d}", [C, C], F32, kind="ExternalInput")
           for d in range(2)]
    wt_d = [nc.dram_tensor(f"wt{d}", [C], F32, kind="ExternalInput")
            for d in range(2)]
    wv_d = [nc.dram_tensor(f"wv{d}", [C, C], F32, kind="ExternalInput")
            for d in range(2)]
    bv_d = [nc.dram_tensor(f"bv{d}", [C], F32, kind="ExternalInput")
            for d in range(2)]
    w1_d = nc.dram_tensor("w1", [2 * C, C], F32, kind="ExternalInput")
    b1_d = nc.dram_tensor("b1", [C], F32, kind="ExternalInput")
    w2_d = nc.dram_tensor("w2", [C, C], F32, kind="ExternalInput")
    b2_d = nc.dram_tensor("b2", [C], F32, kind="ExternalInput")
    out_d = nc.dram_tensor("out", [C, 2], F32, kind="ExternalOutput")

    with tile.TileContext(nc) as tc, nc.allow_low_precision(reason="f32r/fp8"):
        with (
            tc.tile_pool(name="const", bufs=1) as const,
            tc.tile_pool(name="seq", bufs=1) as seqp,
            tc.tile_pool(name="tt", bufs=1) as ttp,
            tc.tile_pool(name="vv", bufs=1) as vvp,
            tc.tile_pool(name="ee", bufs=2) as eep,
            tc.tile_pool(name="small", bufs=1) as smallp,
            tc.tile_pool(name="s6", bufs=1, space="PSUM") as s6p,
            tc.tile_pool(name="pk", bufs=1, space="PSUM") as pkp,
            tc.tile_pool(name="scr", bufs=1, space="PSUM") as scrp,
        ):
            # --- sequence + weight loads (d=0 weights first: needed early) ---
            xq_sbs = [seqp.tile([128, 2, N], F32R, tag=f"xq{b}",
                                name=f"xq_sb{b}") for b in range(2)]
            xk_sbs = [seqp.tile([128, 2, N], F32R, tag=f"xk{b}",
                                name=f"xk_sb{b}") for b in range(2)]
            one_sb = const.tile([128, 1], F32)
            nc.vector.memset(one_sb, 1.0)
            zero1 = const.tile([128, 1], F32)
            nc.vector.memset(zero1, 0.0)

            a_sb, wt_sb, wv_sb, bv_sb = [], [], [], []
            for d in range(2):
                a = const.tile([128, 2, C], F32R, tag=f"a{d}")
                nc.sync.dma_start(
                    out=a,
                    in_=a_d[d].ap().rearrange("(k p) c -> p k c", p=128).bitcast(F32R))
                a_sb.append(a)
                wt = const.tile([128, 2], F32, tag=f"wt{d}")
                nc.sync.dma_start(
                    out=wt, in_=wt_d[d].ap().rearrange("(t p) -> p t", p=128))
                wt_sb.append(wt)
                wv = const.tile([128, 2, C], F32R, tag=f"wv{d}")
                nc.scalar.dma_start(
                    out=wv,
                    in_=wv_d[d].ap().rearrange("(k p) c -> p k c", p=128).bitcast(F32R))
                wv_sb.append(wv)
                bv = const.tile([1, C], F32, tag=f"bv{d}")
                nc.scalar.dma_start(
                    out=bv, in_=bv_d[d].ap().rearrange("(o c) -> o c", o=1))
                bv_sb.append(bv)
                if d == 0:
                    nc.sync.dma_start(
                        out=xq_sbs[0],
                        in_=xq_d.ap()[0].rearrange(
                            "(k p) n -> p k n", p=128).bitcast(F32R))
                    nc.scalar.dma_start(
                        out=xk_sbs[0],
                        in_=xk_d.ap()[0].rearrange(
                            "(k p) n -> p k n", p=128).bitcast(F32R))
                    nc.sync.dma_start(
                        out=xq_sbs[1],
                        in_=xq_d.ap()[1].rearrange(
                            "(k p) n -> p k n", p=128).bitcast(F32R))
                    nc.scalar.dma_start(
                        out=xk_sbs[1],
                        in_=xk_d.ap()[1].rearrange(
                            "(k p) n -> p k n", p=128).bitcast(F32R))

            w1_sb = const.tile([128, 4, C], F32)
            nc.sync.dma_start(
                out=w1_sb, in_=w1_d.ap().rearrange("(k p) c -> p k c", p=128))
            b1_sb = const.tile([128, 2], F32)
            nc.sync.dma_start(
                out=b1_sb, in_=b1_d.ap().rearrange("(t p) -> p t", p=128))
            w2_sb = const.tile([128, 2, C], F32)
            nc.scalar.dma_start(
                out=w2_sb, in_=w2_d.ap().rearrange("(k p) c -> p k c", p=128))
            b2_sb = const.tile([128, 2], F32)
            nc.scalar.dma_start(
                out=b2_sb, in_=b2_d.ap().rearrange("(t p) -> p t", p=128))

            # fp8 copies of the sequences (kv-side scores operand), x1/16.
            xq8s = [seqp.tile([128, 2, N], FP8, tag=f"xq8{b}",
                              name=f"xq8_{b}") for b in range(2)]
            xk8s = [seqp.tile([128, 2, N], FP8, tag=f"xk8{b}",
                              name=f"xk8_{b}") for b in range(2)]
            nc.vector.tensor_scalar_mul(
                xk8s[0], xk_sbs[0].bitcast(F32), K_CAST)
            nc.gpsimd.tensor_scalar_mul(
                xq8s[0], xq_sbs[0].bitcast(F32), K_CAST)
            nc.gpsimd.tensor_scalar_mul(
                xk8s[1], xk_sbs[1].bitcast(F32), K_CAST)
            nc.gpsimd.tensor_scalar_mul(
                xq8s[1], xq_sbs[1].bitcast(F32), K_CAST)

            ft_sb = const.tile([128, 8], F32)  # fused^T columns (k-chunk, b)

            # scores ring: three 2-bank tiles; pieces cycle tiles by
            # global half-block index so PE writes ~1.5 blocks ahead of exp
            s3 = [s6p.tile([128, 1024], F32, tag=f"s{k}", name=f"s3_{k}")
                  for k in range(3)]

            seq_idx = [(r, b, d) for r in range(reps)
                       for b in range(2) for d in range(2)]
            n_idx = len(seq_idx)

            # ---- startup: T~ and V for the four (b,d) combos (both are
            # rep-invariant, so no per-rep PSUM traffic or evacuation) ----
            tt_tiles = {}
            v_tiles = {}
            for b in range(2):
                for d in range(2):
                    q = xq_sbs[b] if d == 0 else xk_sbs[b]
                    kv = xk_sbs[b] if d == 0 else xq_sbs[b]
                    aa, ww, wvv = a_sb[d], wt_sb[d], wv_sb[d]
                    t = ttp.tile([128, 2, N], FP8, tag=f"tt{b}{d}",
                                 name=f"tt{b}{d}")
                    tt_tiles[(b, d)] = t
                    v = vvp.tile([128, BLOCKS, C], BF16, tag=f"v{b}{d}",
                                 name=f"v{b}{d}")
                    v_tiles[(b, d)] = v
                    for ct in range(2):
                        for j4 in range(4):
                            lo = 512 * j4
                            ps = (pkp if j4 % 2 == 0 else scrp).tile(
                                [128, 512], F32,
                                tag="pacc" if j4 % 2 == 0 else "scr0",
                                name=f"ttps{b}{d}_{ct}{j4}")
                            nc.tensor.matmul(
                                ps, aa[:, 0, ct * 128:(ct + 1) * 128],
                                q[:, 0, lo:lo + 512], start=True, stop=False)
                            nc.tensor.matmul(
                                ps, aa[:, 1, ct * 128:(ct + 1) * 128],
                                q[:, 1, lo:lo + 512], start=False, stop=True)
                            if j4 % 2 == 0:
                                nc.scalar.activation(
                                    t[:, ct, lo:lo + 512], ps, AF.Identity,
                                    bias=ww[:, ct:ct + 1], scale=1.0)
                            else:
                                nc.vector.tensor_scalar_add(
                                    t[:, ct, lo:lo + 512], ps,
                                    ww[:, ct:ct + 1])
                    for mb in range(BLOCKS):
                        ps = (pkp if mb % 2 == 0 else scrp).tile(
                            [128, 512], F32,
                            tag="pacc" if mb % 2 == 0 else "scr0",
                            name=f"vps{b}{d}_{mb}")
                        nc.tensor.matmul(
                            ps[:, :C], kv[:, 0, mb * 128:(mb + 1) * 128],
                            wvv[:, 0, :], start=True, stop=False)
                        nc.tensor.matmul(
                            ps[:, :C], kv[:, 1, mb * 128:(mb + 1) * 128],
                            wvv[:, 1, :], start=False, stop=True)
                        if mb % 2 == 0:
                            nc.scalar.activation(
                                v[:, mb, :], ps[:, :C], AF.Identity)
                        else:
                            nc.vector.tensor_copy(v[:, mb, :], ps[:, :C])

            pair_tiles = {}   # (gi, pair) -> (e0, e1, rv)
            block_data = {}   # g -> [partial, partial]
            paccs = {}        # gi -> (chunk tiles, started[4])

            def ctx(gi):
                _, b, d = seq_idx[gi]
                k8 = xk8s[b] if d == 0 else xq8s[b]
                kv = xk_sbs[b] if d == 0 else xq_sbs[b]
                return b, d, k8, kv

            def scores_half(g, half):
                gi, nb = divmod(g, BLOCKS)
                _, b, d = seq_idx[gi]
                _, _, k8, _ = ctx(gi)
                tt = tt_tiles[(b, d)]
                for ci in ((0, 1) if half == 0 else (2, 3)):
                    t3 = s3[(2 * g + ci // 2) % 3]
                    nc.tensor.matmul(
                        t3[:, (ci % 2) * 512:(ci % 2 + 1) * 512],
                        tt[:, :, nb * 128:(nb + 1) * 128],
                        k8[:, :, ci * 512:(ci + 1) * 512],
                        start=True, stop=True, perf_mode=DR)

            def emit_exp(g):
                gi, nb = divmod(g, BLOCKS)
                pair, par = nb >> 1, nb & 1
                key = (gi, pair)
                if par == 0:
                    e0 = eep.tile([128, 2, 1024], BF16, tag="e16a",
                                  name=f"e16a_{gi}_{pair}", bufs=4)
                    if (pair, 1) in D_COLS:
                        e1 = eep.tile([128, 2, 1024], I16, tag="e16d",
                                      name=f"e16d_{gi}_{pair}", bufs=4)
                    else:
                        e1 = eep.tile([128, 2, 1024], BF16, tag="e16b",
                                      name=f"e16b_{gi}_{pair}", bufs=3)
                    rv_t = smallp.tile([128, 2, 1], BF16, tag="rv16", bufs=4)
                    pair_tiles[key] = (e0, e1, rv_t)
                e0, e1, rv_t = pair_tiles[key]

                parts = []
                for pc, e_t in ((0, e0), (1, e1)):
                    sc = s3[(2 * g + pc) % 3]
                    if (pair, pc) not in D_COLS:
                        ra = smallp.tile([128, 1], F32, tag=f"ra{pc}", bufs=6)
                        nc.scalar.activation(
                            e_t[:, par, :], sc, AF.Exp,
                            scale=1.0 / EXP_SCALE, accum_out=ra)
                    else:
                        nc.vector.tensor_scalar_add(
                            e_t[:, par, :], sc, EXP_BIAS)
                        ra = smallp.tile([128, 1], F32, tag=f"ra{pc}", bufs=6)
                        d_rowsums.append((g, e_t, par, ra))
                    parts.append(ra)
                block_data[g] = parts

            def d_rowsum_adds(g, e_t, par):
                t1 = smallp.tile([128, 512], BF16, tag="dr1", bufs=3)
                nc.gpsimd.tensor_add(
                    t1, e_t[:, par, 0:512].bitcast(BF16),
                    e_t[:, par, 512:1024].bitcast(BF16))
                t2 = smallp.tile([128, 256], BF16, tag="dr2", bufs=3)
                nc.gpsimd.tensor_add(t2, t1[:, 0:256], t1[:, 256:512])
                return t2

            def d_rowsum_reduce(t2, ra):
                nc.vector.tensor_reduce(
                    ra, t2, axis=mybir.AxisListType.X,
                    op=mybir.AluOpType.add)

            def emit_colsum(gi, pair):
                e0, e1, rv_t = pair_tiles.pop((gi, pair))
                if gi not in paccs:
                    paccs[gi] = (pkp.tile([128, 512], F32, tag="pacc",
                                          name=f"pacc{gi}"), [False] * 4)
                pacc, started = paccs[gi]
                for par in range(2):
                    pa, pb = block_data.pop(BLOCKS * gi + 2 * pair + par)
                    rs = smallp.tile([128, 1], F32, tag="rs", bufs=4)
                    nc.gpsimd.tensor_add(rs, pa, pb)
                    nc.vector.reciprocal(rv_t[:, par, :], rs)
                last = pair == BLOCKS // 2 - 1
                for ci in range(4):
                    st = not started[ci]
                    started[ci] = True
                    pc, off = ci // 2, (ci % 2) * 512
                    e_t = (e0, e1)[pc]
                    dd = (pair, pc) in D_COLS
                    for pp in range(2):
                        nc.tensor.matmul(
                            pacc[32 * ci:32 * ci + 1, :],
                            rv_t[:, pp, :],
                            e_t[:, pp, off:off + 512].bitcast(BF16)
                            if dd else e_t[:, pp, off:off + 512],
                            start=st and pp == 0,
                            stop=last and pp == 1,
                            tile_position=(0, 32 * ci),
                            skip_group_check=True)

            tail_state = {}

            def tail_p(gi):
                pacc, _ = paccs.pop(gi)
                p_sb = smallp.tile([1, 4, 512], F32, tag="p",
                                   name=f"p{gi}", bufs=1)
                for ci in range(4):
                    if ci % 2 == 0:
                        nc.scalar.activation(
                            p_sb[:, ci, :], pacc[32 * ci:32 * ci + 1, :],
                            AF.Identity)
                    else:
                        nc.vector.tensor_copy(
                            p_sb[:, ci, :], pacc[32 * ci:32 * ci + 1, :])
                tail_state[gi] = p_sb

            def tail_ptp(gi):
                p_sb = tail_state[gi]
                ptp = scrp.tile([128, 512], F32, tag="scr0", name=f"ptp{gi}")
                for j in range(BLOCKS):
                    nc.tensor.matmul(
                        ptp[:, j:j + 1],
                        p_sb[0:1, j // 4, (j % 4) * 128:(j % 4 + 1) * 128],
                        one_sb[0:1, :], start=(j == 0),
                        stop=(j == BLOCKS - 1), skip_group_check=True)
                pt_sb = smallp.tile([128, 16], BF16, tag="pt", bufs=2)
                nc.vector.tensor_copy(pt_sb, ptp[:, :16])
                tail_state[gi] = pt_sb

            def tail_yps(gi):
                pt_sb = tail_state[gi]
                b, d, _, _ = ctx(gi)
                v = v_tiles[(b, d)]
                yps = scrp.tile([128, 512], F32, tag="scr0", name=f"yps{gi}")
                for j in range(BLOCKS):
                    nc.tensor.matmul(
                        yps[0:1, :C], pt_sb[:, j:j + 1], v[:, j, :],
                        start=(j == 0), stop=(j == BLOCKS - 1),
                        skip_group_check=True)
                y_sb = smallp.tile([1, C], F32, tag="y", bufs=1)
                nc.vector.tensor_add(y_sb, yps[0:1, :C], bv_sb[d])
                tail_state[gi] = y_sb

            def tail_fin(gi):
                y_sb = tail_state.pop(gi)
                _, b, d = seq_idx[gi]
                fcol = scrp.tile([128, 512], F32, tag="scr0", name=f"fcol{gi}")
                for h in range(2):
                    nc.tensor.matmul(
                        fcol[:, h:h + 1], y_sb[0:1, h * 128:(h + 1) * 128],
                        one_sb[0:1, :], start=(h == 0), stop=(h == 1),
                        skip_group_check=True)
                for h in range(2):
                    k = 2 * d + h
                    nc.vector.tensor_copy(
                        ft_sb[:, 2 * k + b:2 * k + b + 1], fcol[:, h:h + 1])

            schedule = {}
            d_rowsums = []     # (g, e_t, par, ra) awaiting pool adds
            d_reduces = []     # (t2, ra) awaiting DVE reduce

            def at(git, fn):
                schedule.setdefault(git, []).append(fn)

            for gi in range(n_idx):
                for p in range(8):
                    at(BLOCKS * gi + 2 * p + 5,
                       lambda gi=gi, p=p: emit_colsum(gi, p))
                base = BLOCKS * gi + BLOCKS + 3
                at(base, lambda gi=gi: tail_p(gi))
                at(base + 1, lambda gi=gi: tail_ptp(gi))
                at(base + 2, lambda gi=gi: tail_yps(gi))
                at(base + 3, lambda gi=gi: tail_fin(gi))

            total = BLOCKS * n_idx
            for git in range(total + 9):
                if git < total:
                    scores_half(git, 0)
                for fn in schedule.pop(git, []):
                    fn()
                if git < total:
                    scores_half(git, 1)
                if 1 <= git <= total:
                    emit_exp(git - 1)
                # deferred D rowsums: pool adds one iteration after the
                # exp, DVE reduce the iteration after that
                while d_reduces:
                    t2, ra = d_reduces.pop(0)
                    d_rowsum_reduce(t2, ra)
                while d_rowsums:
                    g0, e_t0, par0, ra0 = d_rowsums.pop(0)
                    d_reduces.append((d_rowsum_adds(g0, e_t0, par0), ra0))

            # --- final MLP on the two local batch rows ---
            h_sb = smallp.tile([128, 2, 2], F32, tag="h")
            for t in range(2):
                hps = scrp.tile([128, 512], F32, tag="scr0", name=f"hps{t}")
                for k in range(4):
                    nc.tensor.matmul(
                        hps[:, 0:2], w1_sb[:, k, t * 128:(t + 1) * 128],
                        ft_sb[:, 2 * k:2 * k + 2],
                        start=(k == 0), stop=(k == 3), skip_group_check=True)
                nc.scalar.activation(
                    h_sb[:, t, :], hps[:, 0:2], AF.Relu,
                    bias=b1_sb[:, t:t + 1], scale=1.0)
            o_sb = smallp.tile([128, 2, 2], F32, tag="o")
            for t in range(2):
                ops = pkp.tile([128, 512], F32, tag="pacc", name=f"ops{t}")
                for k in range(2):
                    nc.tensor.matmul(
                        ops[:, 0:2], w2_sb[:, k, t * 128:(t + 1) * 128],
                        h_sb[:, k, :],
                        start=(k == 0), stop=(k == 1), skip_group_check=True)
                nc.scalar.activation(
                    o_sb[:, t, :], ops[:, 0:2], AF.Identity,
                    bias=b2_sb[:, t:t + 1], scale=1.0)
            nc.sync.dma_start(
                out=out_d.ap().rearrange("(t p) b -> p t b", p=128), in_=o_sb)

    split_multi_waits(nc)
    return nc


_NC = None


def _get_nc():
    global _NC
    if _NC is None:
        _NC = build_nc()
    return _NC


def prep_inputs(stft_feat, cqt_feat, wq1_w, wq1_b, wq2_w, wq2_b, wq3_w, wq3_b,
                wq4_w, wq4_b, wq5_w, wq5_b, wq6_w, wq6_b,
                out1_w, out1_b, out2_w, out2_b):
    B = stft_feat.shape[0]
    s = 1.0 / np.sqrt(np.float32(C))
    f32 = np.float32
    sigma = np.float32(TT_HOST)  # fp8 range balancing; scores come out
    # scaled by EXP_SCALE = TT_HOST * K_CAST (Schraudolph pre-scale)
    A1 = (wq1_w @ wq2_w.T * s * sigma).astype(f32)
    wt1 = (wq2_w @ wq1_b * s * sigma).astype(f32)
    A2 = (wq4_w @ wq5_w.T * s * sigma).astype(f32)
    wt2 = (wq5_w @ wq4_b * s * sigma).astype(f32)
    WV1 = (wq3_w / f32(N)).astype(f32)
    WV2 = (wq6_w / f32(N)).astype(f32)
    common = dict(
        a0=np.ascontiguousarray(A1), a1=np.ascontiguousarray(A2),
        wt0=np.ascontiguousarray(wt1), wt1=np.ascontiguousarray(wt2),
        wv0=np.ascontiguousarray(WV1), wv1=np.ascontiguousarray(WV2),
        bv0=np.ascontiguousarray(wq3_b.astype(f32)),
        bv1=np.ascontiguousarray(wq6_b.astype(f32)),
        w1=np.ascontiguousarray(out1_w.astype(f32)),
        b1=np.ascontiguousarray(out1_b.astype(f32)),
        w2=np.ascontiguousarray(out2_w.astype(f32)),
        b2=np.ascontiguousarray(out2_b.astype(f32)),
    )
    stft = np.ascontiguousarray(stft_feat.reshape(B, C, N).astype(f32))
    cqt = np.ascontiguousarray(cqt_feat.reshape(B, C, N).astype(f32))
    in_maps = []
    for i in range(8):
        m = dict(common)
        m["xq"] = np.ascontiguousarray(stft[2 * i:2 * i + 2])
        m["xk"] = np.ascontiguousarray(cqt[2 * i:2 * i + 2])
        in_maps.append(m)
    return in_maps


def kernel(**inputs):
    inputs = {k: np.asarray(v) for k, v in inputs.items()}
    B = inputs["stft_feat"].shape[0]
    nc = _get_nc()
    in_maps = prep_inputs(**inputs)
    res = run_bass_kernel_spmd(nc, in_maps, list(range(8)))
    out = np.empty((B, C), np.float32)
    for i in range(8):
        o = res.results[i]["out"]  # [C, 2]
        out[2 * i] = o[:, 0]
        out[2 * i + 1] = o[:, 1]
    return out
